# revision 19
# baseline (speedup 1.0000x reference)
"""Multi-head attention (B=8, N=1024, D=512, H=8) on 8 TRN2 NeuronCores.

Sharding: pure batch-parallel - core i computes batch i end-to-end, no
collectives. Host-side prep per batch: gather valid keys (mask) into a
contiguous buffer padded to NKV=640, pre-transpose x, convert streams to
bf16, and pack every stream so it loads with ONE dma_start (DMA issue on
the shared HWDGE costs ~630ns per instruction and the per-queue transfer
rate is ~200GB/s, so the 8 loads are spread across the three HWDGE
queues: SP, ACT and DVE).

Device pipeline (bf16 matmuls, f32 PSUM accumulation), fully interleaved
so the PE stream never idles (the Tensor engine clock ramps to 2x after
~3us of continuous execution; gaps reset it):
  k/q projections for the first head pair, then scores immediately; the
  remaining k/q/v projections are interleaved one unit at a time into
  the score/exp stream. Scores for a head pair land in one [128,1024]
  PSUM tile per (chunk, query-half) with the two heads on disjoint PE
  row tiles (concurrent matmuls). exp on ACT folds the key-padding mask
  into the activation bias. attn@v runs in TWO PASSES per head pair
  (query-half 0 heads, then half 1) into a [65, 2, 512] PSUM tile whose
  row 64 collects the softmax denominator via an augmented ones-column
  of v; the next pair's scores are emitted between passes so the PE
  keeps streaming while the normalize chain (denominator copy, fast
  reciprocal, gpsimd partition-broadcast, multiply fused with the
  PSUM->SBUF copy) drains. Out-projection columns 0:512 interleave with
  the final AV pass; output is stored bf16 and upcast on the host.

PSUM budget (8 banks): shared projection/outproj pool 2 + score tiles 4
+ AV accumulator 2.

Math shortcuts: bk is dropped (constant-in-key terms cancel in softmax);
bv is folded into the output bias on the host (bob' = bo + bv @ wo since
normalized attention rows sum to 1). fp8 attn@v was tried and rejected:
v quantization alone puts the max-abs tail at 2.5e-2 (tol 2e-2) and exp
overflows e4m3.
"""

import sys

import numpy as np

sys.path.insert(0, "/opt/trn_rl_repo")

B, N, D, H = 8, 1024, 512, 8
HD = D // H            # 64
SCALE = HD ** -0.5     # 0.125
NKV = 640              # padded valid-key count (5 chunks of 128)
KC = NKV // 128        # 5
DC = D // 128          # 4
VW = HD + 2            # 66: aug head stride, 4B-aligned for bf16 weights
PAD_BIAS = -30000.0    # exp(PAD_BIAS + s*SCALE) == 0.0 exactly

_prog_cache = {}


def _build_program():
    import concourse.bacc as bacc
    import concourse.tile as tile
    from concourse import mybir

    dt = mybir.dt
    f32 = dt.float32
    bf16 = dt.bfloat16
    AF = mybir.ActivationFunctionType

    nc = bacc.Bacc("TRN2", target_bir_lowering=False, debug=False)

    xT_d = nc.dram_tensor("xT", [128, DC, N], bf16, kind="ExternalInput").ap()
    xkT_d = nc.dram_tensor("xkT", [128, DC, NKV], bf16,
                           kind="ExternalInput").ap()
    wq_d = nc.dram_tensor("wq", [128, DC, D], bf16, kind="ExternalInput").ap()
    wk_d = nc.dram_tensor("wk", [128, DC, D], bf16, kind="ExternalInput").ap()
    wv_d = nc.dram_tensor("wv", [128, DC, D], bf16, kind="ExternalInput").ap()
    wo_d = nc.dram_tensor("wo", [128, DC, D], bf16, kind="ExternalInput").ap()
    tbl_d = nc.dram_tensor("tbl", [128, DC + KC], f32,
                           kind="ExternalInput").ap()
    bob_d = nc.dram_tensor("bob", [128, D], f32, kind="ExternalInput").ap()
    y_d = nc.dram_tensor("y", [N, D], bf16, kind="ExternalOutput").ap()

    with tile.TileContext(nc) as tc, \
         nc.allow_low_precision(reason="bf16 matmul streams, f32 accum"), \
         tc.tile_pool(name="const", bufs=1) as cpool, \
         tc.tile_pool(name="kqvpp", bufs=1, space="PSUM") as kqvpp, \
         tc.tile_pool(name="scp", bufs=2, space="PSUM") as scp, \
         tc.tile_pool(name="oap", bufs=1, space="PSUM") as oap, \
         tc.tile_pool(name="pp", bufs=20) as pp, \
         tc.tile_pool(name="dnp", bufs=6) as dnp, \
         tc.tile_pool(name="rbp", bufs=4) as rbp, \
         tc.tile_pool(name="ysp", bufs=2) as ysp:

        wk_t = cpool.tile([128, DC, D], bf16, name="wk_t")
        wq_t = cpool.tile([128, DC, D], bf16, name="wq_t")
        wv_t = cpool.tile([128, DC, D], bf16, name="wv_t")
        wo_t = cpool.tile([128, DC, D], bf16, name="wo_t")
        xkT_t = cpool.tile([128, DC, NKV], bf16, name="xkT_t")
        xT_t = cpool.tile([128, DC, N], bf16, name="xT_t")
        kT_t = cpool.tile([128, DC, NKV], bf16, name="kT_t")
        qT_t = cpool.tile([128, DC, N], bf16, name="qT_t")
        vaug_t = [cpool.tile([128, H, VW], bf16, name=f"vaug_t{c}")
                  for c in range(KC)]
        aoT_t = cpool.tile([128, DC, N], bf16, name="aoT_t")
        tbl_t = cpool.tile([128, DC + KC], f32, name="tbl_t")
        bob_t = cpool.tile([128, D], f32, name="bob_t")

        # Input DMAs spread over the two HWDGE queues (SP, ACT) plus the
        # gpsimd SWDGE queue so transfers run concurrently; per-queue
        # issue order = need order. kproj's inputs lead the fast queues.
        nc.sync.dma_start(xkT_t[:], xkT_d[:])
        nc.sync.dma_start(wk_t[:], wk_d[:])
        nc.scalar.dma_start(tbl_t[:], tbl_d[:])
        nc.scalar.dma_start(wq_t[:], wq_d[:])
        nc.scalar.dma_start(xT_t[:], xT_d[:])
        nc.gpsimd.dma_start(wv_t[:], wv_d[:])
        nc.gpsimd.dma_start(bob_t[:], bob_d[:])
        nc.gpsimd.dma_start(wo_t[:], wo_d[:])

        # ones column of the augmented v (denominator trick)
        for c in range(KC):
            nc.vector.memset(vaug_t[c][:, :, HD:HD + 1], 1.0)

        # ---------- emitter units ----------
        def K(dp):
            ps = kqvpp.tile([128, N], f32, name="kqv_ps", tag="kqv")
            for dc in range(DC):
                lhs = wk_t[:, dc, 128 * dp:128 * (dp + 1)]
                nc.tensor.matmul(ps[:, 0:512], lhs, xkT_t[:, dc, 0:512],
                                 start=(dc == 0), stop=(dc == DC - 1))
                nc.tensor.matmul(ps[:, 512:NKV], lhs, xkT_t[:, dc, 512:NKV],
                                 start=(dc == 0), stop=(dc == DC - 1))
            nc.vector.tensor_scalar_add(kT_t[:, dp, :], ps[:, 0:NKV], 0.0)

        def Q(dp):
            ps = kqvpp.tile([128, N], f32, name="kqv_ps", tag="kqv")
            for dc in range(DC):
                lhs = wq_t[:, dc, 128 * dp:128 * (dp + 1)]
                for hf in range(2):
                    nc.tensor.matmul(ps[:, 512 * hf:512 * (hf + 1)], lhs,
                                     xT_t[:, dc, 512 * hf:512 * (hf + 1)],
                                     start=(dc == 0), stop=(dc == DC - 1))
            nc.vector.tensor_scalar_add(qT_t[:, dp, :], ps[:],
                                        tbl_t[:, dp:dp + 1])

        def V(c):
            ps = kqvpp.tile([128, H, HD], f32, name="kqv_vps", tag="kqv")
            for dc in range(DC):
                nc.tensor.matmul(ps[:],
                                 xkT_t[:, dc, 128 * c:128 * (c + 1)],
                                 wv_t[:, dc, :],
                                 start=(dc == 0), stop=(dc == DC - 1))
            nc.vector.tensor_scalar_add(vaug_t[c][:, :, 0:HD], ps[:], 0.0)

        p_t = {}   # (dp, c, hf) -> [128, N] bf16 exp tile

        def S(dp, c, hf):
            sc = scp.tile([128, N], f32, name="sc")
            for hi in range(2):
                row = HD * hi
                nc.tensor.matmul(
                    sc[:, 512 * hi:512 * (hi + 1)],
                    kT_t[row:row + HD, dp, 128 * c:128 * (c + 1)],
                    qT_t[row:row + HD, dp, 512 * hf:512 * (hf + 1)],
                    start=True, stop=True)
            p = pp.tile([128, N], bf16, name="p")
            nc.scalar.activation(p[:], sc[:], AF.Exp,
                                 bias=tbl_t[:, DC + c:DC + c + 1], scale=SCALE)
            p_t[(dp, c, hf)] = p

        oa_t = {}  # (dp, hf) -> [65, 2, 512] PSUM accumulator

        def A(dp, hf, c):
            if c == 0:
                oa_t[(dp, hf)] = oap.tile([HD + 1, 2, 512], f32, name="oa2")
            oa = oa_t[(dp, hf)]
            p = p_t[(dp, c, hf)]
            for hi in range(2):
                nc.tensor.matmul(
                    oa[:, hi, :], vaug_t[c][:, 2 * dp + hi, 0:HD + 1],
                    p[:, 512 * hi:512 * (hi + 1)],
                    start=(c == 0), stop=(c == KC - 1))

        def CH(dp, hf):
            oa = oa_t[(dp, hf)]
            for hi in range(2):
                db = dnp.tile([1, 512], f32, name="db")
                nc.vector.tensor_scalar_add(db[:], oa[HD:HD + 1, hi, :], 0.0)
                rc = dnp.tile([1, 512], f32, name="rc")
                nc.vector.reciprocal_approx_fast(rc[:], db[:])
                rbs = rbp.tile([HD, 512], f32, name="rbs")
                nc.gpsimd.partition_broadcast(rbs[:], rc[:])
                row = HD * hi
                nc.vector.tensor_mul(
                    aoT_t[row:row + HD, dp, 512 * hf:512 * (hf + 1)],
                    oa[0:HD, hi, :], rbs[:])

        def OP(ic):
            yps = kqvpp.tile([128, D], f32, name="yps", tag="kqv")
            for dp in range(DC):
                nc.tensor.matmul(yps[:], aoT_t[:, dp, 128 * ic:128 * (ic + 1)],
                                 wo_t[:, dp, :],
                                 start=(dp == 0), stop=(dp == DC - 1))
            ysb = ysp.tile([128, D], bf16, name="ysb")
            nc.vector.tensor_add(ysb[:], yps[:], bob_t[:])
            nc.sync.dma_start(y_d[128 * ic:128 * (ic + 1), :], ysb[:])

        # ---------- schedule ----------
        # Filler units (projections) are drip-fed into the score/exp
        # stream; AV passes for (dp, hf) are separated by the next
        # pair's scores so the oa2 WAR wait (normalize chain reads)
        # never stalls the PE queue head. `done` asserts emission-order
        # dependencies at build time.
        done = set()

        def S_(dp, c, hf):
            assert ("kq", dp) in done, (dp, c, hf)
            S(dp, c, hf)
            done.add(("s", dp, c, hf))

        def A_(dp, hf, c):
            assert ("s", dp, c, hf) in done and ("v", c) in done, (dp, hf, c)
            A(dp, hf, c)

        def K_(dp):
            K(dp)

        def Q_(dp):
            Q(dp)
            done.add(("kq", dp))

        def V_(c):
            V(c)
            done.add(("v", c))

        K_(0); Q_(0)
        S_(0, 0, 0); S_(0, 0, 1); V_(0)
        S_(0, 1, 0); S_(0, 1, 1); V_(1); A_(0, 0, 0)
        S_(0, 2, 0); S_(0, 2, 1); V_(2); A_(0, 0, 1)
        S_(0, 3, 0); S_(0, 3, 1); V_(3); K_(1); A_(0, 0, 2)
        S_(0, 4, 0); S_(0, 4, 1); V_(4); Q_(1); A_(0, 0, 3); A_(0, 0, 4)
        CH(0, 0)
        S_(1, 0, 0); S_(1, 0, 1); K_(2)
        S_(1, 1, 0); S_(1, 1, 1); Q_(2)
        A_(0, 1, 0); A_(0, 1, 1); S_(1, 2, 0); A_(0, 1, 2); S_(1, 2, 1)
        A_(0, 1, 3); A_(0, 1, 4)
        CH(0, 1)
        S_(1, 3, 0); S_(1, 3, 1); K_(3)
        A_(1, 0, 0); A_(1, 0, 1); S_(1, 4, 0); A_(1, 0, 2); S_(1, 4, 1)
        A_(1, 0, 3); A_(1, 0, 4)
        CH(1, 0)
        S_(2, 0, 0); S_(2, 0, 1); Q_(3)
        A_(1, 1, 0); A_(1, 1, 1); S_(2, 1, 0); A_(1, 1, 2); S_(2, 1, 1)
        A_(1, 1, 3); A_(1, 1, 4)
        CH(1, 1)
        S_(2, 2, 0); S_(2, 2, 1)
        A_(2, 0, 0); A_(2, 0, 1); S_(2, 3, 0); A_(2, 0, 2); S_(2, 3, 1)
        A_(2, 0, 3); S_(2, 4, 0); A_(2, 0, 4)
        CH(2, 0)
        S_(2, 4, 1); S_(3, 0, 0)
        A_(2, 1, 0); A_(2, 1, 1); S_(3, 0, 1); A_(2, 1, 2); S_(3, 1, 0)
        A_(2, 1, 3); A_(2, 1, 4)
        CH(2, 1)
        S_(3, 1, 1); S_(3, 2, 0)
        A_(3, 0, 0); A_(3, 0, 1); S_(3, 2, 1); A_(3, 0, 2); S_(3, 3, 0)
        A_(3, 0, 3); S_(3, 4, 0); A_(3, 0, 4)
        CH(3, 0)
        S_(3, 3, 1); S_(3, 4, 1)
        A_(3, 1, 0); OP(0); A_(3, 1, 1); OP(1); A_(3, 1, 2); OP(2)
        A_(3, 1, 3); OP(3); A_(3, 1, 4)
        # Late out-proj (ic 4..7): accumulate the dp0-2 contributions into
        # retired score-pool slots while the dp3 normalize chain drains;
        # only the dp3 matmul + bias + store remain after the chain.
        yab = []
        for j in range(2):
            t = scp.tile([128, 2, 512], f32, name="yab", tag="sc")
            for icp in range(2):
                ic = 4 + 2 * j + icp
                for dp in range(3):
                    nc.tensor.matmul(
                        t[:, icp, :], aoT_t[:, dp, 128 * ic:128 * (ic + 1)],
                        wo_t[:, dp, :], start=(dp == 0), stop=False)
            yab.append(t)
        CH(3, 1)
        for j in range(2):
            for icp in range(2):
                ic = 4 + 2 * j + icp
                nc.tensor.matmul(
                    yab[j][:, icp, :], aoT_t[:, 3, 128 * ic:128 * (ic + 1)],
                    wo_t[:, 3, :], start=False, stop=True)
                ysb = ysp.tile([128, D], bf16, name="ysb")
                nc.vector.tensor_add(ysb[:], yab[j][:, icp, :], bob_t[:])
                nc.sync.dma_start(y_d[128 * ic:128 * (ic + 1), :], ysb[:])

    return nc


def _get_program():
    if "nc" not in _prog_cache:
        nc = _build_program()
        if not nc.is_finalized():
            nc.finalize()
        _prog_cache["nc"] = nc
    return _prog_cache["nc"]


def _packT(m):
    """[R, C] -> [128, R//128, C] so one DMA fills a [128, R//128 * C] tile."""
    r, c = m.shape
    return np.ascontiguousarray(
        m.reshape(r // 128, 128, c).transpose(1, 0, 2))


def _prep_core(b, x, mask, wq, bq, wk, bk, wv, bv, wo, bo):
    import ml_dtypes

    b16 = ml_dtypes.bfloat16
    f = np.float32
    xb = np.ascontiguousarray(x[b], dtype=f)                # [N, D]
    idx = np.nonzero(mask[b])[0]
    nv = int(idx.size)
    assert 1 <= nv <= NKV, f"batch {b}: {nv} valid keys, NKV={NKV}"
    xk = np.zeros((NKV, D), f)
    xk[:nv] = xb[idx]
    pos = np.arange(128)[:, None] + 128 * np.arange(KC)[None, :]
    expb = np.where(pos < nv, 0.0, PAD_BIAS).astype(f)      # [128, KC]
    tbl = np.concatenate(
        [np.ascontiguousarray(bq, f).reshape(DC, 128).T, expb], axis=1)
    bob = (bo.astype(f) + bv.astype(f) @ wo.astype(f)).reshape(D)
    return {
        "xT": _packT(np.ascontiguousarray(xb.T)).astype(b16),
        "xkT": _packT(np.ascontiguousarray(xk.T)).astype(b16),
        "wq": _packT(np.ascontiguousarray(wq, f)).astype(b16),
        "wk": _packT(np.ascontiguousarray(wk, f)).astype(b16),
        "wv": _packT(np.ascontiguousarray(wv, f)).astype(b16),
        "wo": _packT(np.ascontiguousarray(wo, f)).astype(b16),
        "tbl": np.ascontiguousarray(tbl),
        "bob": np.ascontiguousarray(np.broadcast_to(bob, (128, D))),
    }


def _run(inputs):
    import os

    os.environ["BASS_NEVER_TRACE"] = "1"
    from concourse.bass_utils import run_bass_kernel_spmd

    nc = _get_program()
    in_maps = [_prep_core(b, **inputs) for b in range(B)]
    res = run_bass_kernel_spmd(nc, in_maps, core_ids=list(range(B)),
                               trace=False)
    out = np.stack([res.results[b]["y"] for b in range(B)], axis=0)
    return out.astype(np.float32), res


def kernel(**inputs) -> np.ndarray:
    out, _ = _run(inputs)
    return out
